# revision 19
# baseline (speedup 1.0000x reference)
"""Trainium2 Bass kernel for nn_BERTHeading (conv1/2/3 over tokens + max-pool heads).

Strategy: pure data-parallel over 8 NeuronCores (batch 4096 -> 512/core).
Per core:
  - All six conv taps (W1[0], W2[0], W2[1], W3[0], W3[1], W3[2]) are treated as
    one big 768->512 matmul bank; token-position shifts are realised as shifted
    views of the transposed activation matrix accumulated in PSUM.
  - Layout: features on partitions (4 chunks of 128), rows = (batch*20) in the
    free dimension.  words_emb is transposed on-chip with PE-transpose.
  - Matmuls in bf16 (fp32 accumulate); L2 norms via an all-ones [128,128]
    stationary matmul (cross-partition sum + broadcast in one shot), ScalarE
    Sqrt and a fast DVE reciprocal.
"""

import os
import sys

for _p in ("/opt/trn_rl_repo", "/root/.axon_site/_ro/trn_rl_repo", "/root/.axon_site"):
    if os.path.isdir(_p) and _p not in sys.path:
        sys.path.append(_p)

import numpy as np
import ml_dtypes

import concourse.bass as bass
import concourse.mybir as mybir
import concourse.tile as tile
from concourse import bacc
from concourse.bass_utils import run_bass_kernel_spmd
from concourse.masks import make_identity

F32 = mybir.dt.float32
BF16 = mybir.dt.bfloat16
AF = mybir.ActivationFunctionType

N_CORES = 8
SEQ = 20
EMB = 768
FEAT = 512
KC = EMB // 128   # 6 contraction chunks
FC = FEAT // 128  # 4 feature chunks
RCB = 25          # batches per row-chunk (25*20 = 500 rows <= 512 psum bank)
EPS2 = 1e-24      # (1e-12)^2, matches F.normalize eps under the sqrt

# tap index -> (weight slot, position shift)
TAPS_A = [(0, 0)]
TAPS_B = [(1, 0), (2, 1)]
TAPS_C = [(3, 0), (4, 1), (5, 2)]

LAST_EXEC_TIME_NS = None


def _install_ntff_hook():
    """Provide antenv.axon_hooks (absent in this image) so trace=True can
    capture NTFF profiles through libaxon_pjrt.so, and neutralize the
    bucket upload which has no credentials here."""
    import types
    try:
        import antenv
        if "antenv.axon_hooks" not in sys.modules:
            from trn_agent_boot.trn_boot import _ntff_profile_via_ctypes
            mod = types.ModuleType("antenv.axon_hooks")
            _state = {"hook": None}
            mod.set_axon_ntff_profile_hook = lambda h: _state.__setitem__("hook", h)
            mod.get_axon_ntff_profile_hook = lambda: _state["hook"]
            sys.modules["antenv.axon_hooks"] = mod
            antenv.axon_hooks = mod
            mod.set_axon_ntff_profile_hook(_ntff_profile_via_ctypes("/opt/axon/libaxon_pjrt.so"))
        import concourse.bass_utils as _bu
        _bu.upload_artifacts = lambda tmpdir: str(tmpdir)
    except Exception as e:  # profiling is best-effort
        print("ntff hook install failed:", e, file=sys.stderr)


def _chunks(total, step):
    out = []
    o = 0
    while o < total:
        out.append((o, min(step, total - o)))
        o += step
    return out


def build(nb_core):
    """Build the per-core Bass graph for nb_core batches per core."""
    nc = bacc.Bacc(None, target_bir_lowering=False)

    x_d = nc.declare_dram_parameter("words_emb", [nb_core, SEQ, EMB], F32, isOutput=False)
    se_d = nc.declare_dram_parameter("sent_emb", [nb_core, EMB], F32, isOutput=False)
    wt_d = nc.declare_dram_parameter("wt", [128, FC, 6, KC, 128], BF16, isOutput=False)
    wp_d = nc.declare_dram_parameter("wp", [128, KC, FEAT], BF16, isOutput=False)
    b1_d = nc.declare_dram_parameter("b1", [FEAT], F32, isOutput=False)
    b2_d = nc.declare_dram_parameter("b2", [FEAT], F32, isOutput=False)
    b3_d = nc.declare_dram_parameter("b3", [FEAT], F32, isOutput=False)
    bp_d = nc.declare_dram_parameter("bp", [FEAT], F32, isOutput=False)

    # device layouts: feature chunk on partitions, contiguous per-partition runs;
    # host permutes to [b, f, l] / [b, f] during the gather step
    wo_d = nc.declare_dram_parameter("words_out", [128, FC, nb_core * SEQ], F32, isOutput=True)
    wv_d = nc.declare_dram_parameter("word_vector", [128, FC, nb_core], F32, isOutput=True)
    so_d = nc.declare_dram_parameter("sent", [nb_core, FEAT], F32, isOutput=True)

    x_flat = x_d.ap().rearrange("b l d -> (b l) d")

    row_chunks = _chunks(nb_core, RCB)     # (batch0, nbatches)
    sent_chunks = _chunks(nb_core, 128)    # (b0, nb) over sent rows

    from contextlib import ExitStack
    with tile.TileContext(nc) as tc, ExitStack() as ctx:
        singles = ctx.enter_context(tc.tile_pool(name="singles", bufs=1))
        xsp = ctx.enter_context(tc.tile_pool(name="xsp", bufs=12))
        xtp = ctx.enter_context(tc.tile_pool(name="xtp", bufs=3))
        abcp = ctx.enter_context(tc.tile_pool(name="abcp", bufs=6))
        codep = ctx.enter_context(tc.tile_pool(name="codep", bufs=6))
        sqp = ctx.enter_context(tc.tile_pool(name="sqp", bufs=2))
        invp = ctx.enter_context(tc.tile_pool(name="invp", bufs=2))
        outp = ctx.enter_context(tc.tile_pool(name="outp", bufs=6))
        smallp = ctx.enter_context(tc.tile_pool(name="smallp", bufs=2))
        ptap = ctx.enter_context(tc.tile_pool(name="ptap", bufs=4, space=bass.MemorySpace.PSUM))
        ptr = ctx.enter_context(tc.tile_pool(name="ptr", bufs=2, space=bass.MemorySpace.PSUM))
        pssp = ctx.enter_context(tc.tile_pool(name="pssp", bufs=1, space=bass.MemorySpace.PSUM))
        pssvp = ctx.enter_context(tc.tile_pool(name="pssvp", bufs=1, space=bass.MemorySpace.PSUM))

        # ---- constants ----
        ident = singles.tile([128, 128], F32, tag="ident")
        make_identity(nc, ident)
        ones_bf = singles.tile([128, 128], BF16, tag="ones")
        nc.vector.memset(ones_bf, 1.0)
        eps_b = singles.tile([128, 1], F32, tag="epsb")
        nc.vector.memset(eps_b, EPS2)
        zero_b = singles.tile([128, 1], F32, tag="zerob")
        nc.vector.memset(zero_b, 0.0)
        # first ACT instruction uses Sqrt so the table-set picker settles on
        # sqrt_and_others (contains relu/square/sqrt/copy) once, up front
        actdummy = singles.tile([128, 1], F32, tag="actdummy")
        nc.scalar.activation(actdummy, eps_b, AF.Sqrt, bias=zero_b)

        wpsb = singles.tile([128, KC, FEAT], BF16, tag="wpsb")
        nc.scalar.dma_start(out=wpsb, in_=wp_d.ap())
        wsb = singles.tile([128, FC, 6, KC, 128], BF16, tag="wsb")
        for f in range(FC):
            nc.gpsimd.dma_start(out=wsb[:, f], in_=wt_d[:, f])

        bsb = []
        for nm, bd in (("b1", b1_d), ("b2", b2_d), ("b3", b3_d)):
            t = singles.tile([128, FC], F32, tag=nm)
            nc.scalar.dma_start(out=t, in_=bd.ap().rearrange("(c p) -> p c", p=128))
            bsb.append(t)

        bp_ap = bp_d.ap()
        bp_bcast = singles.tile([128, FEAT], F32, tag="bpb")
        nc.gpsimd.dma_start(
            out=bp_bcast,
            in_=bass.AP(tensor=bp_ap.tensor, offset=bp_ap.offset, ap=[[0, 128], [1, FEAT]]),
        )

        wv_all = singles.tile([128, FC, nb_core], F32, tag="wv_all")

        # ---- helper: natural-layout load + PE transpose into [128, KC, R] bf16 ----
        def load_transposed(src_rows_ap, r0, R, parity):
            """Return bf16 SBUF tile [128, KC, 512] holding src[r0:r0+R, :].T."""
            nblk = (R + 127) // 128
            xs_tiles = []
            for blk in range(nblk):
                nr = min(128, R - blk * 128)
                xs = xsp.tile([128, EMB], F32, tag="xs")
                nc.sync.dma_start(out=xs[:nr, :], in_=src_rows_ap[r0 + blk * 128: r0 + blk * 128 + nr, :])
                xs_tiles.append((xs, nr))
            xT = xtp.tile([128, KC, 512], BF16, tag="xt")
            for c in range(KC):
                pt = ptr.tile([128, 4, 128], F32, tag="tr")
                for blk, (xs, nr) in enumerate(xs_tiles):
                    nc.tensor.transpose(pt[:, blk, :nr], xs[:nr, c * 128:(c + 1) * 128], ident[:nr, :nr])
                ptf = pt.rearrange("p a b -> p (a b)")
                if (c + parity) % 2 == 0:
                    nc.vector.tensor_copy(xT[:, c, 0:R], ptf[:, 0:R])
                else:
                    nc.scalar.copy(xT[:, c, 0:R], ptf[:, 0:R])
            return xT

        # ============== sent head first: PE warm-up while weights load ==============
        seT = load_transposed(se_d.ap(), 0, nb_core, 0)  # [128, KC, nb_core] bf16
        for (s0, nsb) in sent_chunks:
            pso = ptap.tile([128, 512], F32, tag="tap")
            for k in range(KC):
                nc.tensor.matmul(pso[:nsb, :FEAT], seT[:, k, s0:s0 + nsb], wpsb[:, k, :],
                                 start=(k == 0), stop=(k == KC - 1), skip_group_check=True)
            so_sb = outp.tile([128, FEAT], F32, tag="sosb")
            nc.vector.tensor_add(so_sb[:nsb, :], pso[:nsb, :FEAT], bp_bcast[:nsb, :])
            nc.scalar.dma_start(out=so_d[s0:s0 + nsb, :], in_=so_sb[:nsb, :])

        # =========================== main loop ===========================
        PF = 2
        xT_q = [load_transposed(x_flat, row_chunks[i][0] * SEQ, row_chunks[i][1] * SEQ, i)
                for i in range(min(PF, len(row_chunks)))]
        for rc, (b0, nb) in enumerate(row_chunks):
            R = nb * SEQ
            RB = nb
            xT = xT_q.pop(0)
            if rc + PF < len(row_chunks):
                nb2 = row_chunks[rc + PF][1]
                xT_q.append(load_transposed(x_flat, row_chunks[rc + PF][0] * SEQ, nb2 * SEQ, rc + PF))

            pss = pssp.tile([128, 512], F32, tag="ss")
            pooled = smallp.tile([128, FC, 32], F32, tag="pooled")
            psq = smallp.tile([128, FC, 32], BF16, tag="psq")
            pssv = pssvp.tile([128, 32], F32, tag="ssv")
            on_big = outp.tile([128, FC, 512], F32, tag="onorm")
            code_tiles = []

            for fc in range(FC):
                # ---- conv taps: matmuls with shifted rhs views ----
                pa = ptap.tile([128, 512], F32, tag="tap")
                pb = ptap.tile([128, 512], F32, tag="tap")
                pc = ptap.tile([128, 512], F32, tag="tap")
                for grp, taps in ((pa, TAPS_A), (pb, TAPS_B), (pc, TAPS_C)):
                    n = len(taps) * KC
                    i = 0
                    for (t, s) in taps:
                        for k in range(KC):
                            nc.tensor.matmul(
                                grp[:, 0:R - s],
                                wsb[:, fc, t, k, :],
                                xT[:, k, s:R],
                                start=(i == 0),
                                stop=(i == n - 1),
                                skip_group_check=True,
                            )
                            i += 1

                # ---- relu(+bias) -> bf16 SBUF ----
                a_sb = abcp.tile([128, 512], BF16, tag="abc")
                b_sb = abcp.tile([128, 512], BF16, tag="abc")
                c_sb = abcp.tile([128, 512], BF16, tag="abc")
                nc.scalar.activation(a_sb[:, :R], pa[:, :R], AF.Relu, bias=bsb[0][:, fc:fc + 1])
                nc.scalar.activation(b_sb[:, :R], pb[:, :R], AF.Relu, bias=bsb[1][:, fc:fc + 1])
                nc.scalar.activation(c_sb[:, :R], pc[:, :R], AF.Relu, bias=bsb[2][:, fc:fc + 1])
                a3 = a_sb[:, :R].rearrange("p (b l) -> p b l", l=SEQ)
                b19 = b_sb[:, :R].rearrange("p (b l) -> p b l", l=SEQ)[:, :, 0:19]
                c18 = c_sb[:, :R].rearrange("p (b l) -> p b l", l=SEQ)[:, :, 0:18]

                # ---- positional max over the three convs (ReLU>=0 padding) ----
                code = codep.tile([128, 512], BF16, tag="code")
                code3 = code[:, :R].rearrange("p (b l) -> p b l", l=SEQ)
                nc.vector.tensor_max(code3[:, :, 0:19], a3[:, :, 0:19], b19)
                nc.vector.tensor_copy(code3[:, :, 19:20], a3[:, :, 19:20])
                nc.vector.tensor_max(code3[:, :, 0:18], code3[:, :, 0:18], c18)
                code_tiles.append(code)

                # ---- sum of squares over features (ones-matmul broadcast) ----
                sq = sqp.tile([128, 512], BF16, tag="sq")
                nc.scalar.activation(sq[:, :R], code[:, :R], AF.Square, bias=zero_b)
                nc.tensor.matmul(pss[:, :R], ones_bf, sq[:, :R],
                                 start=(fc == 0), stop=(fc == FC - 1), skip_group_check=True)

                # ---- per-conv global max over positions -> pooled ----
                pm = smallp.tile([128, 3, 32], F32, tag="pm")
                nc.vector.reduce_max(pm[:, 0, :RB], a3, axis=mybir.AxisListType.X)
                nc.vector.reduce_max(pm[:, 1, :RB], b19, axis=mybir.AxisListType.X)
                nc.vector.reduce_max(pm[:, 2, :RB], c18, axis=mybir.AxisListType.X)
                pmt = smallp.tile([128, 32], F32, tag="pmt")
                nc.vector.tensor_add(pmt[:, :RB], pm[:, 0, :RB], pm[:, 1, :RB])
                nc.vector.tensor_add(pmt[:, :RB], pmt[:, :RB], pm[:, 2, :RB])
                nc.vector.tensor_scalar_mul(pooled[:, fc, :RB], pmt[:, :RB], 1.0 / 3.0)
                nc.scalar.activation(psq[:, fc, :RB], pooled[:, fc, :RB], AF.Square, bias=zero_b)
                nc.tensor.matmul(pssv[:, :RB], ones_bf, psq[:, fc, :RB],
                                 start=(fc == 0), stop=(fc == FC - 1), skip_group_check=True)

            # ---- normalize code -> words_out (contiguous device layout) ----
            inv = invp.tile([128, 512], F32, tag="inv")
            sn = invp.tile([128, 512], F32, tag="sn")
            nc.scalar.activation(sn[:, :R], pss[:, :R], AF.Sqrt, bias=eps_b)
            nc.vector.reciprocal_approx_fast(out=inv[:, :R], in_=sn[:, :R])
            for fc in range(FC):
                nc.vector.tensor_mul(on_big[:, fc, :R], code_tiles[fc][:, :R], inv[:, :R])
            nc.scalar.dma_start(out=wo_d[:, :, b0 * SEQ: b0 * SEQ + R], in_=on_big[:, :, :R])

            # ---- normalize pooled -> word_vector staging ----
            invv = smallp.tile([128, 32], F32, tag="invv")
            snv = smallp.tile([128, 32], F32, tag="snv")
            nc.scalar.activation(snv[:, :RB], pssv[:, :RB], AF.Sqrt, bias=eps_b)
            nc.vector.reciprocal_approx_fast(out=invv[:, :RB], in_=snv[:, :RB])
            for fc in range(FC):
                nc.vector.tensor_mul(wv_all[:, fc, b0:b0 + nb], pooled[:, fc, :RB], invv[:, :RB])

        # word_vector out, device layout [128, FC, nb]
        nc.scalar.dma_start(out=wv_d.ap(), in_=wv_all)

    nc.compile()
    if not nc.is_finalized():
        nc.finalize()
    return nc


_CACHE = {}


def _get_built(nb_core):
    if nb_core not in _CACHE:
        _CACHE[nb_core] = build(nb_core)
    return _CACHE[nb_core]


def _pack_weights(W1, W2, W3, Wp):
    wt_all = np.stack([W1[0], W2[0], W2[1], W3[0], W3[1], W3[2]])  # [6, 768, 512]
    wt = wt_all.reshape(6, KC, 128, FC, 128).transpose(2, 3, 0, 1, 4)  # [p, f, t, k, m]
    wt = np.ascontiguousarray(wt).astype(ml_dtypes.bfloat16)
    wp = np.ascontiguousarray(Wp.reshape(KC, 128, FEAT).transpose(1, 0, 2)).astype(ml_dtypes.bfloat16)
    return wt, wp


def kernel(words_emb, sent_emb, W1, b1, W2, b2, W3, b3, Wp, bp):
    global LAST_EXEC_TIME_NS
    words_emb = np.ascontiguousarray(np.asarray(words_emb, dtype=np.float32))
    sent_emb = np.ascontiguousarray(np.asarray(sent_emb, dtype=np.float32))
    W1, W2, W3, Wp = (np.asarray(w, np.float32) for w in (W1, W2, W3, Wp))
    b1, b2, b3, bp = (np.ascontiguousarray(np.asarray(b, np.float32)) for b in (b1, b2, b3, bp))

    bs = words_emb.shape[0]
    assert bs % N_CORES == 0
    nb_core = bs // N_CORES

    wt, wp = _pack_weights(W1, W2, W3, Wp)
    nc = _get_built(nb_core)

    in_maps = []
    for c in range(N_CORES):
        sl = slice(c * nb_core, (c + 1) * nb_core)
        in_maps.append({
            "words_emb": words_emb[sl],
            "sent_emb": sent_emb[sl],
            "wt": wt, "wp": wp,
            "b1": b1, "b2": b2, "b3": b3, "bp": bp,
        })

    trace = bool(int(os.environ.get("KERNEL_TRACE", "0")))
    if trace:
        _install_ntff_hook()
    res = run_bass_kernel_spmd(nc, in_maps, core_ids=list(range(N_CORES)), trace=trace)
    LAST_EXEC_TIME_NS = res.exec_time_ns

    # undo the device layouts while gathering shards:
    # words_out dev [128, FC, nb*SEQ] -> [nb, FC*128, SEQ];  wv dev [128, FC, nb] -> [nb, FC*128]
    words_out = np.concatenate([
        r["words_out"].reshape(128, FC, nb_core, SEQ).transpose(2, 1, 0, 3).reshape(nb_core, FEAT, SEQ)
        for r in res.results], axis=0)
    word_vector = np.concatenate([
        r["word_vector"].reshape(128, FC, nb_core).transpose(2, 1, 0).reshape(nb_core, FEAT)
        for r in res.results], axis=0)
    sent = np.concatenate([r["sent"] for r in res.results], axis=0)
    return np.ascontiguousarray(words_out), np.ascontiguousarray(word_vector), sent


if __name__ == "__main__":
    # smoke test with random data
    rng = np.random.default_rng(0)
    bs = int(os.environ.get("SMOKE_BS", "64"))
    ins = {
        "words_emb": rng.standard_normal((bs, SEQ, EMB), np.float32),
        "sent_emb": rng.standard_normal((bs, EMB), np.float32),
        "W1": rng.standard_normal((1, EMB, FEAT), np.float32) * 0.02,
        "b1": np.zeros(FEAT, np.float32),
        "W2": rng.standard_normal((2, EMB, FEAT), np.float32) * 0.02,
        "b2": np.zeros(FEAT, np.float32),
        "W3": rng.standard_normal((3, EMB, FEAT), np.float32) * 0.02,
        "b3": np.zeros(FEAT, np.float32),
        "Wp": rng.standard_normal((EMB, FEAT), np.float32) * 0.02,
        "bp": np.zeros(FEAT, np.float32),
    }
    outs = kernel(**ins)
    for o in outs:
        print(o.shape, o.dtype, float(np.abs(o).mean()))


# revision 20
# speedup vs baseline: 1.0534x; 1.0534x over previous
"""Trainium2 Bass kernel for nn_BERTHeading (conv1/2/3 over tokens + max-pool heads).

Strategy: pure data-parallel over 8 NeuronCores (batch 4096 -> 512/core).
Per core:
  - All six conv taps (W1[0], W2[0], W2[1], W3[0], W3[1], W3[2]) are treated as
    one big 768->512 matmul bank; token-position shifts are realised as shifted
    views of the transposed activation matrix accumulated in PSUM.
  - Layout: features on partitions (4 chunks of 128), rows = (batch*20) in the
    free dimension.  words_emb is transposed on-chip with PE-transpose.
  - Matmuls in bf16 (fp32 accumulate); L2 norms via an all-ones [128,128]
    stationary matmul (cross-partition sum + broadcast in one shot), ScalarE
    Sqrt and a fast DVE reciprocal.
"""

import os
import sys

for _p in ("/opt/trn_rl_repo", "/root/.axon_site/_ro/trn_rl_repo", "/root/.axon_site"):
    if os.path.isdir(_p) and _p not in sys.path:
        sys.path.append(_p)

import numpy as np
import ml_dtypes

import concourse.bass as bass
import concourse.mybir as mybir
import concourse.tile as tile
from concourse import bacc
from concourse.bass_utils import run_bass_kernel_spmd
from concourse.masks import make_identity

F32 = mybir.dt.float32
BF16 = mybir.dt.bfloat16
AF = mybir.ActivationFunctionType

N_CORES = 8
SEQ = 20
EMB = 768
FEAT = 512
KC = EMB // 128   # 6 contraction chunks
FC = FEAT // 128  # 4 feature chunks
RCB = 25          # batches per row-chunk (25*20 = 500 rows <= 512 psum bank)
EPS2 = 1e-24      # (1e-12)^2, matches F.normalize eps under the sqrt

# tap index -> (weight slot, position shift)
TAPS_A = [(0, 0)]
TAPS_B = [(1, 0), (2, 1)]
TAPS_C = [(3, 0), (4, 1), (5, 2)]

LAST_EXEC_TIME_NS = None


def _install_ntff_hook():
    """Provide antenv.axon_hooks (absent in this image) so trace=True can
    capture NTFF profiles through libaxon_pjrt.so, and neutralize the
    bucket upload which has no credentials here."""
    import types
    try:
        import antenv
        if "antenv.axon_hooks" not in sys.modules:
            from trn_agent_boot.trn_boot import _ntff_profile_via_ctypes
            mod = types.ModuleType("antenv.axon_hooks")
            _state = {"hook": None}
            mod.set_axon_ntff_profile_hook = lambda h: _state.__setitem__("hook", h)
            mod.get_axon_ntff_profile_hook = lambda: _state["hook"]
            sys.modules["antenv.axon_hooks"] = mod
            antenv.axon_hooks = mod
            mod.set_axon_ntff_profile_hook(_ntff_profile_via_ctypes("/opt/axon/libaxon_pjrt.so"))
        import concourse.bass_utils as _bu
        _bu.upload_artifacts = lambda tmpdir: str(tmpdir)
    except Exception as e:  # profiling is best-effort
        print("ntff hook install failed:", e, file=sys.stderr)


def _chunks(total, step):
    out = []
    o = 0
    while o < total:
        out.append((o, min(step, total - o)))
        o += step
    return out


def build(nb_core):
    """Build the per-core Bass graph for nb_core batches per core."""
    nc = bacc.Bacc(None, target_bir_lowering=False)

    x_d = nc.declare_dram_parameter("words_emb", [nb_core, SEQ, EMB], F32, isOutput=False)
    se_d = nc.declare_dram_parameter("sent_emb", [nb_core, EMB], F32, isOutput=False)
    wt_d = nc.declare_dram_parameter("wt", [128, FC, 6, KC, 128], BF16, isOutput=False)
    wp_d = nc.declare_dram_parameter("wp", [128, KC, FEAT], BF16, isOutput=False)
    b1_d = nc.declare_dram_parameter("b1", [FEAT], F32, isOutput=False)
    b2_d = nc.declare_dram_parameter("b2", [FEAT], F32, isOutput=False)
    b3_d = nc.declare_dram_parameter("b3", [FEAT], F32, isOutput=False)
    bp_d = nc.declare_dram_parameter("bp", [FEAT], F32, isOutput=False)

    # device layouts: feature chunk on partitions, contiguous per-partition runs;
    # host permutes to [b, f, l] / [b, f] during the gather step
    wo_d = nc.declare_dram_parameter("words_out", [128, FC, nb_core * SEQ], F32, isOutput=True)
    wv_d = nc.declare_dram_parameter("word_vector", [128, FC, nb_core], F32, isOutput=True)
    so_d = nc.declare_dram_parameter("sent", [nb_core, FEAT], F32, isOutput=True)

    x_flat = x_d.ap().rearrange("b l d -> (b l) d")

    row_chunks = _chunks(nb_core, RCB)     # (batch0, nbatches)
    sent_chunks = _chunks(nb_core, 128)    # (b0, nb) over sent rows

    from contextlib import ExitStack
    with tile.TileContext(nc) as tc, ExitStack() as ctx:
        singles = ctx.enter_context(tc.tile_pool(name="singles", bufs=1))
        xsp = ctx.enter_context(tc.tile_pool(name="xsp", bufs=12))
        xtp = ctx.enter_context(tc.tile_pool(name="xtp", bufs=3))
        abcp = ctx.enter_context(tc.tile_pool(name="abcp", bufs=6))
        codep = ctx.enter_context(tc.tile_pool(name="codep", bufs=6))
        sqp = ctx.enter_context(tc.tile_pool(name="sqp", bufs=2))
        invp = ctx.enter_context(tc.tile_pool(name="invp", bufs=2))
        outp = ctx.enter_context(tc.tile_pool(name="outp", bufs=6))
        smallp = ctx.enter_context(tc.tile_pool(name="smallp", bufs=2))
        ptap = ctx.enter_context(tc.tile_pool(name="ptap", bufs=4, space=bass.MemorySpace.PSUM))
        ptr = ctx.enter_context(tc.tile_pool(name="ptr", bufs=2, space=bass.MemorySpace.PSUM))
        pssp = ctx.enter_context(tc.tile_pool(name="pssp", bufs=1, space=bass.MemorySpace.PSUM))
        pssvp = ctx.enter_context(tc.tile_pool(name="pssvp", bufs=1, space=bass.MemorySpace.PSUM))

        # ---- constants ----
        ident = singles.tile([128, 128], BF16, tag="ident")
        make_identity(nc, ident)
        ones_bf = singles.tile([128, 128], BF16, tag="ones")
        nc.vector.memset(ones_bf, 1.0)
        eps_b = singles.tile([128, 1], F32, tag="epsb")
        nc.vector.memset(eps_b, EPS2)
        zero_b = singles.tile([128, 1], F32, tag="zerob")
        nc.vector.memset(zero_b, 0.0)
        # first ACT instruction uses Sqrt so the table-set picker settles on
        # sqrt_and_others (contains relu/square/sqrt/copy) once, up front
        actdummy = singles.tile([128, 1], F32, tag="actdummy")
        nc.scalar.activation(actdummy, eps_b, AF.Sqrt, bias=zero_b)

        wpsb = singles.tile([128, KC, FEAT], BF16, tag="wpsb")
        nc.scalar.dma_start(out=wpsb, in_=wp_d.ap())
        wsb = singles.tile([128, FC, 6, KC, 128], BF16, tag="wsb")
        for f in range(FC):
            nc.scalar.dma_start(out=wsb[:, f], in_=wt_d[:, f])

        bsb = []
        for nm, bd in (("b1", b1_d), ("b2", b2_d), ("b3", b3_d)):
            t = singles.tile([128, FC], F32, tag=nm)
            nc.scalar.dma_start(out=t, in_=bd.ap().rearrange("(c p) -> p c", p=128))
            bsb.append(t)

        wv_all = singles.tile([128, FC, nb_core], F32, tag="wv_all")

        # ---- helper: natural-layout load + PE transpose into [128, KC, R] bf16 ----
        def load_transposed(src_rows_ap, r0, R, parity):
            """Return bf16 SBUF tile [128, KC, 512] holding src[r0:r0+R, :].T.
            The f32->bf16 cast happens inline in the SWDGE DMA."""
            nblk = (R + 127) // 128
            xs_tiles = []
            for blk in range(nblk):
                nr = min(128, R - blk * 128)
                xs = xsp.tile([128, EMB], BF16, tag="xs")
                nc.gpsimd.dma_start(out=xs[:nr, :], in_=src_rows_ap[r0 + blk * 128: r0 + blk * 128 + nr, :])
                xs_tiles.append((xs, nr))
            xT = xtp.tile([128, KC, 512], BF16, tag="xt")
            for c in range(KC):
                pt = ptr.tile([128, 4, 128], BF16, tag="tr")
                for blk, (xs, nr) in enumerate(xs_tiles):
                    nc.tensor.transpose(pt[:, blk, :nr], xs[:nr, c * 128:(c + 1) * 128], ident[:nr, :nr])
                ptf = pt.rearrange("p a b -> p (a b)")
                if (c + parity) % 2 == 0:
                    nc.vector.tensor_copy(xT[:, c, 0:R], ptf[:, 0:R])
                else:
                    nc.scalar.copy(xT[:, c, 0:R], ptf[:, 0:R])
            return xT

        # ============== sent head first: PE warm-up while weights load ==============
        seT = load_transposed(se_d.ap(), 0, nb_core, 0)  # [128, KC, nb_core] bf16
        bp_ap = bp_d.ap()
        bp_bcast = singles.tile([128, FEAT], F32, tag="bpb")
        nc.gpsimd.dma_start(
            out=bp_bcast,
            in_=bass.AP(tensor=bp_ap.tensor, offset=bp_ap.offset, ap=[[0, 128], [1, FEAT]]),
        )
        for (s0, nsb) in sent_chunks:
            pso = ptap.tile([128, 512], F32, tag="tap")
            for k in range(KC):
                nc.tensor.matmul(pso[:nsb, :FEAT], seT[:, k, s0:s0 + nsb], wpsb[:, k, :],
                                 start=(k == 0), stop=(k == KC - 1), skip_group_check=True)
            so_sb = outp.tile([128, FEAT], F32, tag="sosb")
            nc.vector.tensor_add(so_sb[:nsb, :], pso[:nsb, :FEAT], bp_bcast[:nsb, :])
            nc.scalar.dma_start(out=so_d[s0:s0 + nsb, :], in_=so_sb[:nsb, :])

        # =========================== main loop ===========================
        PF = 2
        xT_q = [load_transposed(x_flat, row_chunks[i][0] * SEQ, row_chunks[i][1] * SEQ, i)
                for i in range(min(PF, len(row_chunks)))]
        for rc, (b0, nb) in enumerate(row_chunks):
            R = nb * SEQ
            RB = nb
            xT = xT_q.pop(0)
            if rc + PF < len(row_chunks):
                nb2 = row_chunks[rc + PF][1]
                xT_q.append(load_transposed(x_flat, row_chunks[rc + PF][0] * SEQ, nb2 * SEQ, rc + PF))

            pss = pssp.tile([128, 512], F32, tag="ss")
            pooled = smallp.tile([128, FC, 32], F32, tag="pooled")
            psq = smallp.tile([128, FC, 32], BF16, tag="psq")
            pssv = pssvp.tile([128, 32], F32, tag="ssv")
            on_big = outp.tile([128, FC, 512], F32, tag="onorm")
            code_tiles = []

            for fc in range(FC):
                # ---- conv taps: matmuls with shifted rhs views ----
                pa = ptap.tile([128, 512], F32, tag="tap")
                pb = ptap.tile([128, 512], F32, tag="tap")
                pc = ptap.tile([128, 512], F32, tag="tap")
                for grp, taps in ((pa, TAPS_A), (pb, TAPS_B), (pc, TAPS_C)):
                    n = len(taps) * KC
                    i = 0
                    for (t, s) in taps:
                        for k in range(KC):
                            nc.tensor.matmul(
                                grp[:, 0:R - s],
                                wsb[:, fc, t, k, :],
                                xT[:, k, s:R],
                                start=(i == 0),
                                stop=(i == n - 1),
                                skip_group_check=True,
                            )
                            i += 1

                # ---- relu(+bias) -> bf16 SBUF ----
                a_sb = abcp.tile([128, 512], BF16, tag="abc")
                b_sb = abcp.tile([128, 512], BF16, tag="abc")
                c_sb = abcp.tile([128, 512], BF16, tag="abc")
                nc.scalar.activation(a_sb[:, :R], pa[:, :R], AF.Relu, bias=bsb[0][:, fc:fc + 1])
                nc.scalar.activation(b_sb[:, :R], pb[:, :R], AF.Relu, bias=bsb[1][:, fc:fc + 1])
                nc.scalar.activation(c_sb[:, :R], pc[:, :R], AF.Relu, bias=bsb[2][:, fc:fc + 1])
                a3 = a_sb[:, :R].rearrange("p (b l) -> p b l", l=SEQ)
                b19 = b_sb[:, :R].rearrange("p (b l) -> p b l", l=SEQ)[:, :, 0:19]
                c18 = c_sb[:, :R].rearrange("p (b l) -> p b l", l=SEQ)[:, :, 0:18]

                # ---- positional max over the three convs (ReLU>=0 padding) ----
                code = codep.tile([128, 512], BF16, tag="code")
                code3 = code[:, :R].rearrange("p (b l) -> p b l", l=SEQ)
                nc.vector.tensor_max(code3[:, :, 0:19], a3[:, :, 0:19], b19)
                nc.vector.tensor_copy(code3[:, :, 19:20], a3[:, :, 19:20])
                nc.vector.tensor_max(code3[:, :, 0:18], code3[:, :, 0:18], c18)
                code_tiles.append(code)

                # ---- sum of squares over features (ones-matmul broadcast) ----
                sq = sqp.tile([128, 512], BF16, tag="sq")
                nc.scalar.activation(sq[:, :R], code[:, :R], AF.Square, bias=zero_b)
                nc.tensor.matmul(pss[:, :R], ones_bf, sq[:, :R],
                                 start=(fc == 0), stop=(fc == FC - 1), skip_group_check=True)

                # ---- per-conv global max over positions -> pooled ----
                pm = smallp.tile([128, 3, 32], F32, tag="pm")
                nc.vector.reduce_max(pm[:, 0, :RB], a3, axis=mybir.AxisListType.X)
                nc.vector.reduce_max(pm[:, 1, :RB], b19, axis=mybir.AxisListType.X)
                nc.vector.reduce_max(pm[:, 2, :RB], c18, axis=mybir.AxisListType.X)
                pmt = smallp.tile([128, 32], F32, tag="pmt")
                nc.vector.tensor_add(pmt[:, :RB], pm[:, 0, :RB], pm[:, 1, :RB])
                nc.vector.tensor_add(pmt[:, :RB], pmt[:, :RB], pm[:, 2, :RB])
                nc.vector.tensor_scalar_mul(pooled[:, fc, :RB], pmt[:, :RB], 1.0 / 3.0)
                nc.scalar.activation(psq[:, fc, :RB], pooled[:, fc, :RB], AF.Square, bias=zero_b)
                nc.tensor.matmul(pssv[:, :RB], ones_bf, psq[:, fc, :RB],
                                 start=(fc == 0), stop=(fc == FC - 1), skip_group_check=True)

            # ---- normalize code -> words_out (contiguous device layout) ----
            inv = invp.tile([128, 512], F32, tag="inv")
            sn = invp.tile([128, 512], F32, tag="sn")
            nc.scalar.activation(sn[:, :R], pss[:, :R], AF.Sqrt, bias=eps_b)
            nc.vector.reciprocal_approx_fast(out=inv[:, :R], in_=sn[:, :R])
            for fc in range(FC):
                nc.vector.tensor_mul(on_big[:, fc, :R], code_tiles[fc][:, :R], inv[:, :R])
            nc.scalar.dma_start(out=wo_d[:, :, b0 * SEQ: b0 * SEQ + R], in_=on_big[:, :, :R])

            # ---- normalize pooled -> word_vector staging ----
            invv = smallp.tile([128, 32], F32, tag="invv")
            snv = smallp.tile([128, 32], F32, tag="snv")
            nc.scalar.activation(snv[:, :RB], pssv[:, :RB], AF.Sqrt, bias=eps_b)
            nc.vector.reciprocal_approx_fast(out=invv[:, :RB], in_=snv[:, :RB])
            for fc in range(FC):
                nc.vector.tensor_mul(wv_all[:, fc, b0:b0 + nb], pooled[:, fc, :RB], invv[:, :RB])

        # word_vector out, device layout [128, FC, nb]
        nc.scalar.dma_start(out=wv_d.ap(), in_=wv_all)

    nc.compile()
    if not nc.is_finalized():
        nc.finalize()
    return nc


_CACHE = {}


def _get_built(nb_core):
    if nb_core not in _CACHE:
        _CACHE[nb_core] = build(nb_core)
    return _CACHE[nb_core]


def _pack_weights(W1, W2, W3, Wp):
    wt_all = np.stack([W1[0], W2[0], W2[1], W3[0], W3[1], W3[2]])  # [6, 768, 512]
    wt = wt_all.reshape(6, KC, 128, FC, 128).transpose(2, 3, 0, 1, 4)  # [p, f, t, k, m]
    wt = np.ascontiguousarray(wt).astype(ml_dtypes.bfloat16)
    wp = np.ascontiguousarray(Wp.reshape(KC, 128, FEAT).transpose(1, 0, 2)).astype(ml_dtypes.bfloat16)
    return wt, wp


def kernel(words_emb, sent_emb, W1, b1, W2, b2, W3, b3, Wp, bp):
    global LAST_EXEC_TIME_NS
    words_emb = np.ascontiguousarray(np.asarray(words_emb, dtype=np.float32))
    sent_emb = np.ascontiguousarray(np.asarray(sent_emb, dtype=np.float32))
    W1, W2, W3, Wp = (np.asarray(w, np.float32) for w in (W1, W2, W3, Wp))
    b1, b2, b3, bp = (np.ascontiguousarray(np.asarray(b, np.float32)) for b in (b1, b2, b3, bp))

    bs = words_emb.shape[0]
    assert bs % N_CORES == 0
    nb_core = bs // N_CORES

    wt, wp = _pack_weights(W1, W2, W3, Wp)
    nc = _get_built(nb_core)

    in_maps = []
    for c in range(N_CORES):
        sl = slice(c * nb_core, (c + 1) * nb_core)
        in_maps.append({
            "words_emb": words_emb[sl],
            "sent_emb": sent_emb[sl],
            "wt": wt, "wp": wp,
            "b1": b1, "b2": b2, "b3": b3, "bp": bp,
        })

    trace = bool(int(os.environ.get("KERNEL_TRACE", "0")))
    if trace:
        _install_ntff_hook()
    res = run_bass_kernel_spmd(nc, in_maps, core_ids=list(range(N_CORES)), trace=trace)
    LAST_EXEC_TIME_NS = res.exec_time_ns

    # undo the device layouts while gathering shards:
    # words_out dev [128, FC, nb*SEQ] -> [nb, FC*128, SEQ];  wv dev [128, FC, nb] -> [nb, FC*128]
    words_out = np.concatenate([
        r["words_out"].reshape(128, FC, nb_core, SEQ).transpose(2, 1, 0, 3).reshape(nb_core, FEAT, SEQ)
        for r in res.results], axis=0)
    word_vector = np.concatenate([
        r["word_vector"].reshape(128, FC, nb_core).transpose(2, 1, 0).reshape(nb_core, FEAT)
        for r in res.results], axis=0)
    sent = np.concatenate([r["sent"] for r in res.results], axis=0)
    return np.ascontiguousarray(words_out), np.ascontiguousarray(word_vector), sent


if __name__ == "__main__":
    # smoke test with random data
    rng = np.random.default_rng(0)
    bs = int(os.environ.get("SMOKE_BS", "64"))
    ins = {
        "words_emb": rng.standard_normal((bs, SEQ, EMB), np.float32),
        "sent_emb": rng.standard_normal((bs, EMB), np.float32),
        "W1": rng.standard_normal((1, EMB, FEAT), np.float32) * 0.02,
        "b1": np.zeros(FEAT, np.float32),
        "W2": rng.standard_normal((2, EMB, FEAT), np.float32) * 0.02,
        "b2": np.zeros(FEAT, np.float32),
        "W3": rng.standard_normal((3, EMB, FEAT), np.float32) * 0.02,
        "b3": np.zeros(FEAT, np.float32),
        "Wp": rng.standard_normal((EMB, FEAT), np.float32) * 0.02,
        "bp": np.zeros(FEAT, np.float32),
    }
    outs = kernel(**ins)
    for o in outs:
        print(o.shape, o.dtype, float(np.abs(o).mean()))
